# revision 19
# baseline (speedup 1.0000x reference)
"""BitLinear (ternary-weight linear + per-row int8 fake-quant) on 8 TRN2 cores.

Reference computation:
    w_mean = mean(|W|);  W_t = sign(W) * (|W| > w_mean)
    s_t    = 127 / (max_i |x[t,i]| + 1e-8)
    x_q    = round(x * s_t) / s_t
    out    = x_q @ (W_t * weight_scale).T          # [8192,2048] @ [2048,8192]

Device strategy (2x4 grid over 8 cores):
  - tokens split in halves (r in {0,1}), out_features split in quarters (c in {0..3})
  - each core: x_half [4096,2048] (f32), W_quarter [2048,2048] (f32)
  - global mean(|W|): each core reduces a DISTINCT 1/8 of W rows, scalar AllReduce.
  - n = round(x*s) is an integer in [-127,127]  -> exact in bf16
    W_t in {-1,0,1}                              -> exact in bf16/fp8
    => matmul with fp32 PSUM accumulation is EXACT integer arithmetic for the
    bf16 chunks; the per-token scale (weight_scale / s_t) applies in the
    epilogue.
  - split-K fp8: the first KB8 of KC k-chunks run as fp8e4 DoubleRow matmuls
    (2 fp8 weights/cell -> 2x PE rate). e4m3 has a 3-bit mantissa so n rounds
    to ulp<=8 there; measured on the real inputs this costs max rel err
    1.48e-2 (KB8=8) vs the 2e-2 gate; the other chunks stay exact bf16.
  - round-half-even == jnp.round via the fp32 magic-number trick (+1.5*2^23).
  - K (in_features) must sit on SBUF partitions for the PE. x tiles are
    transposed ON the PE (identity matmuls) -- that work fills the PE during
    the otherwise-dead mean/AllReduce prelude and the PSUM->SBUF copy doubles
    as the bf16->fp8 convert. W is ternarized then transposed via DMA x-bar
    (SP queue carries ONLY these + x loads, so the blocked-on-mean W
    transposes never head-of-line-block PE-side x work).
  - output is stored bf16 (bf16 rounding adds <=2^-9 rel) and upcast on host.
"""

import functools
from contextlib import ExitStack

import numpy as np

import concourse.bass as bass
import concourse.bass_isa as bass_isa
import concourse.mybir as mybir
import concourse.tile as tile
from concourse import bacc
from concourse.masks import make_identity
from concourse.bass_utils import run_bass_kernel_spmd

P = 128
MAGIC = 12582912.0  # 1.5 * 2**23: forces round-to-nearest-even at integer granularity

F32 = mybir.dt.float32
BF16 = mybir.dt.bfloat16
FP8 = mybir.dt.float8e4
X = mybir.AxisListType.X
ALU = mybir.AluOpType
ACTF = mybir.ActivationFunctionType
DR = mybir.MatmulPerfMode.DoubleRow


def _bitlinear_body(tc, xs, ws, wm, wsc, out, *, KC, MT, NB, WT_TILES, WM_TILES,
                    NB_FREE, n_cores, total_w_elems, KB8, N_PRE):
    nc = tc.nc
    in_dim = KC * P
    out_sh = NB * NB_FREE
    KCB = KC - KB8          # bf16 chunks
    assert KB8 % 2 == 0

    with ExitStack() as ctx:
        consts = ctx.enter_context(tc.tile_pool(name="consts", bufs=1))
        wres = ctx.enter_context(tc.tile_pool(name="wres", bufs=1))
        f32p = ctx.enter_context(tc.tile_pool(name="f32p", bufs=3))
        wldp = ctx.enter_context(tc.tile_pool(name="wldp", bufs=5))
        bfp = ctx.enter_context(tc.tile_pool(name="bfp", bufs=3))
        xqp = ctx.enter_context(tc.tile_pool(name="xqp", bufs=2 * N_PRE + 1))
        outp = ctx.enter_context(tc.tile_pool(name="outp", bufs=3))
        smalls = ctx.enter_context(tc.tile_pool(name="smalls", bufs=3))
        psum = ctx.enter_context(tc.tile_pool(name="psum", bufs=4, space="PSUM"))
        dram = ctx.enter_context(tc.tile_pool(name="dram", bufs=1, space="DRAM"))

        # ---------- Phase 1a: global mean(|W|): 1/8 W shard reduce + AllReduce.
        # Everything here gates ternarize -> wT -> all matmuls, so it runs at
        # scheduler priority 0; wm rows come in as two batched 2-tile DMAs per
        # queue (SP + Pool) to dodge per-DMA semaphore pacing.
        with tc.high_priority():
            acc = consts.tile([P, WM_TILES], F32)
            wm_last = None
            for i in range(WM_TILES):
                wtl = f32p.tile([P, in_dim], F32, tag="wmld", bufs=2,
                                name=f"wm_{i}")
                wm_last = nc.scalar.dma_start(wtl, wm[i * P:(i + 1) * P, :])
                nc.vector.tensor_reduce(acc[:, i:i + 1], wtl, axis=X, op=ALU.add,
                                        apply_absolute_value=True)
            rowsum = consts.tile([P, 1], F32)
            nc.vector.tensor_reduce(rowsum, acc, axis=X, op=ALU.add)
            ones_p = consts.tile([P, 1], F32)
            nc.vector.memset(ones_p, 1.0)
            ones_f = consts.tile([1, P], F32)
            nc.vector.memset(ones_f, 1.0)
            # partition-dim reduce via PE: [1,1] = ones[128,1].T @ rowsum[128,1]
            ps_sum = psum.tile([1, 1], F32, tag="ps", name="ps_sum")
            nc.tensor.matmul(ps_sum, ones_p, rowsum)
            allsum1 = consts.tile([1, 1], F32)
            nc.scalar.copy(allsum1, ps_sum)
            cc_in = dram.tile([1, 1], F32)
            cc_out = dram.tile([1, 1], F32, addr_space="Shared")
            nc.gpsimd.dma_start(cc_in, allsum1)
            nc.gpsimd.collective_compute(
                "AllReduce", ALU.add,
                replica_groups=[list(range(n_cores))],
                ins=[cc_in], outs=[cc_out],
            )
            gsum1 = consts.tile([1, 1], F32)
            nc.gpsimd.dma_start(gsum1, cc_out)

        # weight_scale broadcast + constants (no dependence on the collective)
        wsc1 = consts.tile([1, 1], F32)
        nc.gpsimd.dma_start(wsc1, wsc[:, :])
        ps_wsc = psum.tile([P, 1], F32, tag="ps", name="ps_wsc")
        nc.tensor.matmul(ps_wsc, ones_f, wsc1)
        wscb = consts.tile([P, 1], F32)
        nc.scalar.copy(wscb, ps_wsc)
        negmagic = consts.tile([P, 1], F32)
        nc.vector.memset(negmagic, -MAGIC)
        # negated weight_scale/127 for the (negated-ternary) epilogue scale
        nwsc127 = consts.tile([P, 1], F32)
        nc.scalar.mul(nwsc127, wscb, -1.0 / 127.0)
        ident = consts.tile([P, P], BF16)
        make_identity(nc, ident)

        es_all = consts.tile([P, MT], F32)
        wT16 = wres.tile([P, KCB, out_sh], BF16, name="wT16") if KCB else None
        wT8 = wres.tile([P, KB8, out_sh], FP8, name="wT8") if KB8 else None

        # ---------- x-side pipeline pieces -------------------------------
        def x_prep(mt, after=None, ld_sink=None):
            """load -> per-row quantize -> bf16 n -> transpose -> SBUF.
            Prefill tiles transpose on the (otherwise mean-gated) PE; steady
            tiles use one full-KC DMA x-bar transpose on SP. The fp8 chunk
            convert rides the PSUM->SBUF copy (PE path) or its own ACT copy."""
            xt = f32p.tile([P, in_dim], F32, tag="fst", name=f"x_{mt}")
            ld = nc.sync.dma_start(xt, xs[mt * P:(mt + 1) * P, :])
            if after is not None:
                # scheduling-only edge: keep the post-mean DMA window clear
                # for the W drain (HBM is one serialized resource)
                tile.add_dep_helper(ld.ins, after.ins, sync=False,
                                    reason="defer x load behind W drain")
            if ld_sink is not None:
                ld_sink[0] = ld
            mx = smalls.tile([P, 1], F32, tag="mx", name=f"mx_{mt}")
            nc.vector.tensor_reduce(mx, xt, axis=X, op=ALU.max,
                                    apply_absolute_value=True)
            dd = smalls.tile([P, 1], F32, tag="dd", name=f"dd_{mt}")
            nc.vector.tensor_scalar_add(dd, mx, 1e-8)
            rr = smalls.tile([P, 1], F32, tag="rr", name=f"rr_{mt}")
            nc.vector.reciprocal(rr, dd)
            ss = smalls.tile([P, 1], F32, tag="ss", name=f"ss_{mt}")
            nc.vector.tensor_scalar_mul(ss, rr, 127.0)  # s = 127/(max+1e-8)
            # epilogue scale: -(weight_scale * (max+1e-8) / 127)
            nc.vector.tensor_scalar(es_all[:, mt:mt + 1], dd, nwsc127, None,
                                    ALU.mult)
            # n + MAGIC, written in place over the f32 x tile
            nc.vector.tensor_scalar(xt, xt, ss, MAGIC, ALU.mult, ALU.add)
            nq = bfp.tile([P, in_dim], BF16, tag="nq", bufs=2, name=f"nq_{mt}")
            nc.scalar.activation(nq, xt, ACTF.Identity, bias=negmagic)
            xqf = xqp.tile([P, KC, P], BF16, tag="xqf", name=f"xqf_{mt}")
            pst = psum.tile([P, KC, P], BF16, tag="pst", bufs=2,
                            name=f"pst_{mt}")
            for k in range(KC):
                nc.tensor.transpose(pst[:, k, :], nq[:, k * P:(k + 1) * P],
                                    ident)
            nc.scalar.copy(xqf, pst)
            xq8 = None
            if KB8:
                xq8 = xqp.tile([P, KB8, P], FP8, tag="xq8", name=f"xq8_{mt}")
                nc.scalar.copy(xq8, xqf[:, :KB8, :])
            return xqf, xq8

        def x_mm_slice(mt, nb, xqf, xq8):
            """One [128 x NB_FREE] output slice: 12 matmuls + DVE epilogue +
            direct DMA store. Emitted nb-major in groups so nb=0 work for a
            whole group only needs W i-tiles 4*nb..4*nb+3 -- matmuls start as
            soon as the first slice of wT lands instead of waiting for all of
            it."""
            ps = psum.tile([P, NB_FREE], F32, tag="ps", name=f"ps_{mt}_{nb}")
            for j in range(KB8 // 2):
                nc.tensor.matmul(
                    ps, xq8[:, 2 * j:2 * j + 2, :],
                    wT8[:, 2 * j:2 * j + 2,
                        nb * NB_FREE:(nb + 1) * NB_FREE],
                    start=(j == 0), stop=(KCB == 0 and j == KB8 // 2 - 1),
                    perf_mode=DR,
                )
            for kc in range(KCB):
                nc.tensor.matmul(
                    ps, xqf[:, KB8 + kc, :],
                    wT16[:, kc, nb * NB_FREE:(nb + 1) * NB_FREE],
                    start=(KB8 == 0 and kc == 0), stop=(kc == KCB - 1),
                )
            # out = psum * -(weight_scale*(max+1e-8)/127), on DVE
            # (ACT carries nq + transpose copies; a DVE epilogue cannot be
            # head-of-line-blocked by a late x load's nq)
            oslice = outp.tile([P, NB_FREE], BF16, tag="osl", bufs=4,
                               name=f"osl_{mt}_{nb}")
            nc.vector.tensor_scalar(oslice, ps, es_all[:, mt:mt + 1], None,
                                    ALU.mult)
            nc.gpsimd.dma_start(
                out[mt * P:(mt + 1) * P, nb * NB_FREE:(nb + 1) * NB_FREE],
                oslice)

        # ---------- Phase 1b: prefill the x pipeline while the collective
        # round-trips; all its PE transposes sit before any W-gated PE work.
        xq_ready = {}
        last_prefill_ld = [wm_last]
        for mt in range(N_PRE):
            xq_ready[mt] = x_prep(mt, after=wm_last,
                                  ld_sink=last_prefill_ld)

        # mean broadcast (PE ops emitted only after the prefill transposes so
        # the collective wait cannot head-of-line-block them)
        ps_mean = psum.tile([P, 1], F32, tag="ps", name="ps_mean")
        nc.tensor.matmul(ps_mean, ones_f, gsum1)
        meanv = consts.tile([P, 1], F32)
        nc.scalar.mul(meanv, ps_mean, 1.0 / total_w_elems)
        negmeanv = consts.tile([P, 1], F32)
        nc.scalar.mul(negmeanv, meanv, -1.0)

        # ---------- Phase 2: ternarize W quarter -> xbar-transpose into
        # resident wT16 (bf16 chunks) + wT8 (fp8 chunks).
        # NEGATED ternary on a single DVE chain (1 cross-engine wait per op):
        #   a   = (w > mean)            in {0,1}
        #   wtn = (w < -mean) - a       in {-1,0,1} == -W_t
        # The sign flip is folded into the epilogue scale (nwsc127).
        w_loads = []
        for i in range(WT_TILES):
            wtl = wldp.tile([P, in_dim], F32, tag="wld", name=f"w_{i}")
            wl = nc.gpsimd.dma_start(wtl, ws[i * P:(i + 1) * P, :])
            tile.add_dep_helper(wl.ins, wm_last.ins, sync=False,
                                reason="wm (mean chain) owns HBM first")
            w_loads.append(wl)
            a = bfp.tile([P, in_dim], BF16, tag="wa", bufs=2, name=f"wa_{i}")
            nc.vector.tensor_scalar(a, wtl, meanv, None, ALU.is_gt)
            wtn = bfp.tile([P, in_dim], BF16, tag="wc", bufs=2, name=f"wtn_{i}")
            nc.vector.scalar_tensor_tensor(wtn, wtl, negmeanv, a,
                                           op0=ALU.is_lt, op1=ALU.subtract)
            if KCB:
                nc.sync.dma_start_transpose(
                    wT16[:, :, i * P:(i + 1) * P], wtn[:, KB8 * P:in_dim])
            if KB8:
                wtr8 = bfp.tile([P, KB8, P], BF16, tag="wt8", bufs=2,
                                name=f"wtr8_{i}")
                nc.sync.dma_start_transpose(wtr8, wtn[:, :KB8 * P])
                nc.scalar.copy(wT8[:, :, i * P:(i + 1) * P], wtr8)

        # ---------- Phase 3: nb-major grouped steady state. Group g's nb
        # stage only reads wT[..., nb*NB_FREE:...] (W i-tiles 4nb..4nb+3), so
        # the PE starts on nb=0 as soon as those land while the rest of the W
        # pipeline still runs; preps for group g+1 interleave between stages.
        G = N_PRE
        for g0 in range(0, MT, G):
            grp = [mt for mt in range(g0, min(g0 + G, MT))]
            nxts = [mt for mt in range(g0 + G, min(g0 + 2 * G, MT))]
            for nb in range(NB):
                for mt in grp:
                    x_mm_slice(mt, nb, *xq_ready[mt])
                # spread next-group preps across the nb stages
                k = (len(nxts) + NB - 1) // NB
                for nxt in nxts[nb * k:(nb + 1) * k]:
                    xq_ready[nxt] = x_prep(
                        nxt, after=w_loads[9] if nxt >= 2 * G else None)
            for mt in grp:
                xq_ready.pop(mt)


def build_nc(*, tok_sh, in_dim, out_sh, wm_rows, n_cores=8, nb_free=512, kb8=8,
             n_pre=4):
    assert in_dim % P == 0 and tok_sh % P == 0 and out_sh % nb_free == 0
    assert wm_rows % P == 0
    nc = bacc.Bacc("TRN2", target_bir_lowering=False, debug=False,
                   num_devices=n_cores)
    xs = nc.dram_tensor("xs", [tok_sh, in_dim], F32, kind="ExternalInput")
    ws = nc.dram_tensor("ws", [out_sh, in_dim], F32, kind="ExternalInput")
    wm = nc.dram_tensor("wm", [wm_rows, in_dim], F32, kind="ExternalInput")
    wsc = nc.dram_tensor("wsc", [1, 1], F32, kind="ExternalInput")
    out = nc.dram_tensor("out", [tok_sh, out_sh], BF16, kind="ExternalOutput")
    with tile.TileContext(nc) as tc:
        _bitlinear_body(
            tc, xs, ws, wm, wsc, out,
            KC=in_dim // P, MT=tok_sh // P, NB=out_sh // nb_free,
            WT_TILES=out_sh // P, WM_TILES=wm_rows // P, NB_FREE=nb_free,
            n_cores=n_cores, total_w_elems=float(wm_rows * n_cores * in_dim),
            KB8=kb8, N_PRE=n_pre,
        )
    nc.compile()
    return nc


# ------------------------------------------------------------------ full-size
TOK = 8192          # 4*2048 tokens
IN_DIM = 2048
OUT_TOT = 8192
R, C = 2, 4         # token halves x out-feature quarters
TOK_SH = TOK // R
OUT_SH = OUT_TOT // C
WM_ROWS = OUT_TOT // 8


@functools.lru_cache(maxsize=1)
def _full_nc():
    return build_nc(tok_sh=TOK_SH, in_dim=IN_DIM, out_sh=OUT_SH, wm_rows=WM_ROWS)


def make_in_maps(x, weight, weight_scale):
    x = np.ascontiguousarray(np.asarray(x, dtype=np.float32)).reshape(TOK, IN_DIM)
    w = np.ascontiguousarray(np.asarray(weight, dtype=np.float32))
    wsc = np.asarray(weight_scale, dtype=np.float32).reshape(1, 1)
    in_maps = []
    for d in range(8):
        r, c = divmod(d, C)
        in_maps.append({
            "xs": x[r * TOK_SH:(r + 1) * TOK_SH],
            "ws": w[c * OUT_SH:(c + 1) * OUT_SH],
            "wm": w[d * WM_ROWS:(d + 1) * WM_ROWS],
            "wsc": wsc,
        })
    return in_maps


def assemble(results):
    out = np.empty((TOK, OUT_TOT), dtype=np.float32)
    for d in range(8):
        r, c = divmod(d, C)
        out[r * TOK_SH:(r + 1) * TOK_SH, c * OUT_SH:(c + 1) * OUT_SH] = \
            np.asarray(results[d]["out"]).astype(np.float32)
    return out.reshape(4, 2048, OUT_TOT)


def kernel(x, weight, weight_scale):
    nc = _full_nc()
    in_maps = make_in_maps(x, weight, weight_scale)
    res = run_bass_kernel_spmd(nc, in_maps, core_ids=list(range(8)))
    return assemble(res.results)


# revision 29
# speedup vs baseline: 1.3458x; 1.3458x over previous
"""BitLinear (ternary-weight linear + per-row int8 fake-quant) on 8 TRN2 cores.

Reference computation:
    w_mean = mean(|W|);  W_t = sign(W) * (|W| > w_mean)
    s_t    = 127 / (max_i |x[t,i]| + 1e-8)
    x_q    = round(x * s_t) / s_t
    out    = x_q @ (W_t * weight_scale).T          # [8192,2048] @ [2048,8192]

Device strategy (2x4 grid over 8 cores):
  - tokens split in halves (r in {0,1}), out_features split in quarters (c in {0..3})
  - each core: x_half [4096,2048] (f32), W_quarter [2048,2048] (f32)
  - global mean(|W|): each core reduces a DISTINCT 1/8 of W rows, scalar AllReduce.
  - n = round(x*s) is an integer in [-127,127]  -> exact in bf16
    W_t in {-1,0,1}                              -> exact in bf16/fp8
    => matmul with fp32 PSUM accumulation is EXACT integer arithmetic for the
    bf16 chunks; the per-token scale (weight_scale / s_t) applies in the
    epilogue.
  - split-K fp8: the first KB8 of KC k-chunks run as fp8e4 DoubleRow matmuls
    (2 fp8 weights/cell -> 2x PE rate). e4m3 has a 3-bit mantissa so n rounds
    to ulp<=8 there; measured on the real inputs this costs max rel err
    1.48e-2 (KB8=8) vs the 2e-2 gate; the other chunks stay exact bf16.
  - round-half-even == jnp.round via the fp32 magic-number trick (+1.5*2^23).
  - K (in_features) must sit on SBUF partitions for the PE. x tiles are
    transposed ON the PE (identity matmuls) -- that work fills the PE during
    the otherwise-dead mean/AllReduce prelude and the PSUM->SBUF copy doubles
    as the bf16->fp8 convert. W is ternarized then transposed via DMA x-bar
    (SP queue carries ONLY these + x loads, so the blocked-on-mean W
    transposes never head-of-line-block PE-side x work).
  - output is stored bf16 (bf16 rounding adds <=2^-9 rel) and upcast on host.
  - matmuls are emitted nb-major in groups of N_PRE m-tiles: a group's nb
    stage reads only W i-tiles 4nb..4nb+3, so the PE starts on nb=0 as soon
    as the first wT slice lands (~55us) instead of waiting for all 16 i-tiles;
    each [128,512] output slice is scaled on DVE and DMA-stored directly.
    HBM is one serialized ~345GB/s resource, so scheduling-only dep edges
    order the prelude: wm (mean chain) first, then W prefetch + x prefill,
    late x loads resume mid-W-drain.
"""

import functools
from contextlib import ExitStack

import numpy as np

import concourse.bass as bass
import concourse.bass_isa as bass_isa
import concourse.mybir as mybir
import concourse.tile as tile
from concourse import bacc
from concourse.masks import make_identity
from concourse.bass_utils import run_bass_kernel_spmd

P = 128
MAGIC = 12582912.0  # 1.5 * 2**23: forces round-to-nearest-even at integer granularity

F32 = mybir.dt.float32
BF16 = mybir.dt.bfloat16
FP8 = mybir.dt.float8e4
X = mybir.AxisListType.X
ALU = mybir.AluOpType
ACTF = mybir.ActivationFunctionType
DR = mybir.MatmulPerfMode.DoubleRow


def _bitlinear_body(tc, xs, ws, wm, wsc, out, *, KC, MT, NB, WT_TILES, WM_TILES,
                    NB_FREE, n_cores, total_w_elems, KB8, N_PRE):
    nc = tc.nc
    in_dim = KC * P
    out_sh = NB * NB_FREE
    KCB = KC - KB8          # bf16 chunks
    assert KB8 % 2 == 0

    with ExitStack() as ctx:
        consts = ctx.enter_context(tc.tile_pool(name="consts", bufs=1))
        wres = ctx.enter_context(tc.tile_pool(name="wres", bufs=1))
        f32p = ctx.enter_context(tc.tile_pool(name="f32p", bufs=3))
        wldp = ctx.enter_context(tc.tile_pool(name="wldp", bufs=5))
        bfp = ctx.enter_context(tc.tile_pool(name="bfp", bufs=3))
        xqp = ctx.enter_context(tc.tile_pool(name="xqp", bufs=2 * N_PRE + 1))
        outp = ctx.enter_context(tc.tile_pool(name="outp", bufs=3))
        smalls = ctx.enter_context(tc.tile_pool(name="smalls", bufs=3))
        psum = ctx.enter_context(tc.tile_pool(name="psum", bufs=4, space="PSUM"))
        dram = ctx.enter_context(tc.tile_pool(name="dram", bufs=1, space="DRAM"))

        # ---------- Phase 1a: global mean(|W|): 1/8 W shard reduce + AllReduce.
        # Everything here gates ternarize -> wT -> all matmuls, so it runs at
        # scheduler priority 0; wm rows come in as two batched 2-tile DMAs per
        # queue (SP + Pool) to dodge per-DMA semaphore pacing.
        with tc.high_priority():
            acc = consts.tile([P, WM_TILES], F32)
            wm_last = None
            for i in range(WM_TILES):
                wtl = f32p.tile([P, in_dim], F32, tag="wmld", bufs=2,
                                name=f"wm_{i}")
                wm_last = nc.scalar.dma_start(wtl, wm[i * P:(i + 1) * P, :])
                nc.vector.tensor_reduce(acc[:, i:i + 1], wtl, axis=X, op=ALU.add,
                                        apply_absolute_value=True)
            rowsum = consts.tile([P, 1], F32)
            nc.vector.tensor_reduce(rowsum, acc, axis=X, op=ALU.add)
            ones_p = consts.tile([P, 1], F32)
            nc.vector.memset(ones_p, 1.0)
            ones_f = consts.tile([1, P], F32)
            nc.vector.memset(ones_f, 1.0)
            # partition-dim reduce via PE: [1,1] = ones[128,1].T @ rowsum[128,1]
            ps_sum = psum.tile([1, 1], F32, tag="ps", name="ps_sum")
            nc.tensor.matmul(ps_sum, ones_p, rowsum)
            allsum1 = consts.tile([1, 1], F32)
            nc.scalar.copy(allsum1, ps_sum)
            cc_in = dram.tile([1, 1], F32)
            cc_out = dram.tile([1, 1], F32, addr_space="Shared")
            nc.gpsimd.dma_start(cc_in, allsum1)
            nc.gpsimd.collective_compute(
                "AllReduce", ALU.add,
                replica_groups=[list(range(n_cores))],
                ins=[cc_in], outs=[cc_out],
            )
            gsum1 = consts.tile([1, 1], F32)
            nc.gpsimd.dma_start(gsum1, cc_out)

        # weight_scale broadcast + constants (no dependence on the collective)
        wsc1 = consts.tile([1, 1], F32)
        nc.gpsimd.dma_start(wsc1, wsc[:, :])
        ps_wsc = psum.tile([P, 1], F32, tag="ps", name="ps_wsc")
        nc.tensor.matmul(ps_wsc, ones_f, wsc1)
        wscb = consts.tile([P, 1], F32)
        nc.scalar.copy(wscb, ps_wsc)
        negmagic = consts.tile([P, 1], F32)
        nc.vector.memset(negmagic, -MAGIC)
        # negated weight_scale/127 for the (negated-ternary) epilogue scale
        nwsc127 = consts.tile([P, 1], F32)
        nc.scalar.mul(nwsc127, wscb, -1.0 / 127.0)
        ident = consts.tile([P, P], BF16)
        make_identity(nc, ident)

        es_all = consts.tile([P, MT], F32)
        wT16 = wres.tile([P, KCB, out_sh], BF16, name="wT16") if KCB else None
        wT8 = wres.tile([P, KB8, out_sh], FP8, name="wT8") if KB8 else None

        # ---------- x-side pipeline pieces -------------------------------
        def x_prep(mt, after=None, ld_sink=None):
            """load -> per-row quantize -> bf16 n -> transpose -> SBUF.
            Prefill tiles transpose on the (otherwise mean-gated) PE; steady
            tiles use one full-KC DMA x-bar transpose on SP. The fp8 chunk
            convert rides the PSUM->SBUF copy (PE path) or its own ACT copy."""
            xt = f32p.tile([P, in_dim], F32, tag="fst", bufs=2, name=f"x_{mt}")
            ld = nc.sync.dma_start(xt, xs[mt * P:(mt + 1) * P, :])
            if after is not None:
                # scheduling-only edge: keep the post-mean DMA window clear
                # for the W drain (HBM is one serialized resource)
                tile.add_dep_helper(ld.ins, after.ins, sync=False,
                                    reason="defer x load behind W drain")
            if ld_sink is not None:
                ld_sink[0] = ld
            mx = smalls.tile([P, 1], F32, tag="mx", name=f"mx_{mt}")
            nc.vector.tensor_reduce(mx, xt, axis=X, op=ALU.max,
                                    apply_absolute_value=True)
            dd = smalls.tile([P, 1], F32, tag="dd", name=f"dd_{mt}")
            nc.vector.tensor_scalar_add(dd, mx, 1e-8)
            rr = smalls.tile([P, 1], F32, tag="rr", name=f"rr_{mt}")
            nc.vector.reciprocal(rr, dd)
            ss = smalls.tile([P, 1], F32, tag="ss", name=f"ss_{mt}")
            nc.vector.tensor_scalar_mul(ss, rr, 127.0)  # s = 127/(max+1e-8)
            # epilogue scale: -(weight_scale * (max+1e-8) / 127)
            nc.vector.tensor_scalar(es_all[:, mt:mt + 1], dd, nwsc127, None,
                                    ALU.mult)
            # n + MAGIC, written in place over the f32 x tile
            nc.vector.tensor_scalar(xt, xt, ss, MAGIC, ALU.mult, ALU.add)
            nq = bfp.tile([P, in_dim], BF16, tag="nq", bufs=2, name=f"nq_{mt}")
            nc.scalar.activation(nq, xt, ACTF.Identity, bias=negmagic)
            xqf = xqp.tile([P, KC, P], BF16, tag="xqf", name=f"xqf_{mt}")
            pst = psum.tile([P, KC, P], BF16, tag="pst", bufs=2,
                            name=f"pst_{mt}")
            for k in range(KC):
                nc.tensor.transpose(pst[:, k, :], nq[:, k * P:(k + 1) * P],
                                    ident)
            nc.scalar.copy(xqf, pst)
            xq8 = None
            if KB8:
                xq8 = xqp.tile([P, KB8, P], FP8, tag="xq8", name=f"xq8_{mt}")
                nc.scalar.copy(xq8, xqf[:, :KB8, :])
            return xqf, xq8

        def x_mm_slice(mt, nb, g0, ogrp, xqf, xq8):
            """One [128 x NB_FREE] output slice: 12 matmuls + DVE epilogue +
            direct DMA store. Emitted nb-major in groups so nb=0 work for a
            whole group only needs W i-tiles 4*nb..4*nb+3 -- matmuls start as
            soon as the first slice of wT lands instead of waiting for all of
            it."""
            ps = psum.tile([P, NB_FREE], F32, tag="ps", name=f"ps_{mt}_{nb}")
            for j in range(KB8 // 2):
                nc.tensor.matmul(
                    ps, xq8[:, 2 * j:2 * j + 2, :],
                    wT8[:, 2 * j:2 * j + 2,
                        nb * NB_FREE:(nb + 1) * NB_FREE],
                    start=(j == 0), stop=(KCB == 0 and j == KB8 // 2 - 1),
                    perf_mode=DR,
                )
            for kc in range(KCB):
                nc.tensor.matmul(
                    ps, xqf[:, KB8 + kc, :],
                    wT16[:, kc, nb * NB_FREE:(nb + 1) * NB_FREE],
                    start=(KB8 == 0 and kc == 0), stop=(kc == KCB - 1),
                )
            # out = psum * -(weight_scale*(max+1e-8)/127), on DVE
            # (ACT carries nq + transpose copies; a DVE epilogue cannot be
            # head-of-line-blocked by a late x load's nq); the group's slices
            # land in one staging tile, stored with a single batched DMA
            nc.vector.tensor_scalar(ogrp[:, mt - g0, :], ps,
                                    es_all[:, mt:mt + 1], None, ALU.mult)

        # ---------- Phase 1b: prefill the x pipeline while the collective
        # round-trips; all its PE transposes sit before any W-gated PE work.
        xq_ready = {}
        last_prefill_ld = [wm_last]
        for mt in range(N_PRE):
            xq_ready[mt] = x_prep(mt, after=wm_last,
                                  ld_sink=last_prefill_ld)

        # mean broadcast (PE ops emitted only after the prefill transposes so
        # the collective wait cannot head-of-line-block them)
        ps_mean = psum.tile([P, 1], F32, tag="ps", name="ps_mean")
        nc.tensor.matmul(ps_mean, ones_f, gsum1)
        meanv = consts.tile([P, 1], F32)
        nc.scalar.mul(meanv, ps_mean, 1.0 / total_w_elems)
        negmeanv = consts.tile([P, 1], F32)
        nc.scalar.mul(negmeanv, meanv, -1.0)

        # ---------- Phase 2: ternarize W quarter -> xbar-transpose into
        # resident wT16 (bf16 chunks) + wT8 (fp8 chunks).
        # NEGATED ternary on a single DVE chain (1 cross-engine wait per op):
        #   a   = (w > mean)            in {0,1}
        #   wtn = (w < -mean) - a       in {-1,0,1} == -W_t
        # The sign flip is folded into the epilogue scale (nwsc127).
        w_loads = []
        for i in range(WT_TILES):
            wtl = wldp.tile([P, in_dim], F32, tag="wld", name=f"w_{i}")
            wl = nc.gpsimd.dma_start(wtl, ws[i * P:(i + 1) * P, :])
            tile.add_dep_helper(wl.ins, wm_last.ins, sync=False,
                                reason="wm (mean chain) owns HBM first")
            w_loads.append(wl)
            a = bfp.tile([P, in_dim], BF16, tag="wa", bufs=2, name=f"wa_{i}")
            nc.vector.tensor_scalar(a, wtl, meanv, None, ALU.is_gt)
            wtn = bfp.tile([P, in_dim], BF16, tag="wc", bufs=2, name=f"wtn_{i}")
            nc.vector.scalar_tensor_tensor(wtn, wtl, negmeanv, a,
                                           op0=ALU.is_lt, op1=ALU.subtract)
            if KCB:
                nc.sync.dma_start_transpose(
                    wT16[:, :, i * P:(i + 1) * P], wtn[:, KB8 * P:in_dim])
            if KB8:
                wtr8 = bfp.tile([P, KB8, P], BF16, tag="wt8", bufs=2,
                                name=f"wtr8_{i}")
                nc.sync.dma_start_transpose(wtr8, wtn[:, :KB8 * P])
                nc.scalar.copy(wT8[:, :, i * P:(i + 1) * P], wtr8)

        # ---------- Phase 3: nb-major grouped steady state. Group g's nb
        # stage only reads wT[..., nb*NB_FREE:...] (W i-tiles 4nb..4nb+3), so
        # the PE starts on nb=0 as soon as those land while the rest of the W
        # pipeline still runs; preps for group g+1 interleave between stages.
        G = N_PRE
        for g0 in range(0, MT, G):
            grp = [mt for mt in range(g0, min(g0 + G, MT))]
            nxts = [mt for mt in range(g0 + G, min(g0 + 2 * G, MT))]
            for nb in range(NB):
                ogrp = outp.tile([P, len(grp), NB_FREE], BF16, tag="osl",
                                 bufs=2, name=f"osl_{g0}_{nb}")
                for mt in grp:
                    x_mm_slice(mt, nb, g0, ogrp, *xq_ready[mt])
                nc.gpsimd.dma_start(
                    out[g0 * P:(g0 + len(grp)) * P,
                        nb * NB_FREE:(nb + 1) * NB_FREE].rearrange(
                            "(j p) c -> p j c", p=P),
                    ogrp)
                # spread next-group preps across the nb stages
                k = (len(nxts) + NB - 1) // NB
                for nxt in nxts[nb * k:(nb + 1) * k]:
                    xq_ready[nxt] = x_prep(
                        nxt, after=w_loads[9] if nxt >= 2 * G else None)
            for mt in grp:
                xq_ready.pop(mt)


def build_nc(*, tok_sh, in_dim, out_sh, wm_rows, n_cores=8, nb_free=512, kb8=8,
             n_pre=4):
    assert in_dim % P == 0 and tok_sh % P == 0 and out_sh % nb_free == 0
    assert wm_rows % P == 0
    nc = bacc.Bacc("TRN2", target_bir_lowering=False, debug=False,
                   num_devices=n_cores)
    xs = nc.dram_tensor("xs", [tok_sh, in_dim], F32, kind="ExternalInput")
    ws = nc.dram_tensor("ws", [out_sh, in_dim], F32, kind="ExternalInput")
    wm = nc.dram_tensor("wm", [wm_rows, in_dim], F32, kind="ExternalInput")
    wsc = nc.dram_tensor("wsc", [1, 1], F32, kind="ExternalInput")
    out = nc.dram_tensor("out", [tok_sh, out_sh], BF16, kind="ExternalOutput")
    with tile.TileContext(nc) as tc:
        _bitlinear_body(
            tc, xs, ws, wm, wsc, out,
            KC=in_dim // P, MT=tok_sh // P, NB=out_sh // nb_free,
            WT_TILES=out_sh // P, WM_TILES=wm_rows // P, NB_FREE=nb_free,
            n_cores=n_cores, total_w_elems=float(wm_rows * n_cores * in_dim),
            KB8=kb8, N_PRE=n_pre,
        )
    nc.compile()
    return nc


# ------------------------------------------------------------------ full-size
TOK = 8192          # 4*2048 tokens
IN_DIM = 2048
OUT_TOT = 8192
R, C = 2, 4         # token halves x out-feature quarters
TOK_SH = TOK // R
OUT_SH = OUT_TOT // C
WM_ROWS = OUT_TOT // 8


@functools.lru_cache(maxsize=1)
def _full_nc():
    return build_nc(tok_sh=TOK_SH, in_dim=IN_DIM, out_sh=OUT_SH, wm_rows=WM_ROWS)


def make_in_maps(x, weight, weight_scale):
    x = np.ascontiguousarray(np.asarray(x, dtype=np.float32)).reshape(TOK, IN_DIM)
    w = np.ascontiguousarray(np.asarray(weight, dtype=np.float32))
    wsc = np.asarray(weight_scale, dtype=np.float32).reshape(1, 1)
    in_maps = []
    for d in range(8):
        r, c = divmod(d, C)
        in_maps.append({
            "xs": x[r * TOK_SH:(r + 1) * TOK_SH],
            "ws": w[c * OUT_SH:(c + 1) * OUT_SH],
            "wm": w[d * WM_ROWS:(d + 1) * WM_ROWS],
            "wsc": wsc,
        })
    return in_maps


def assemble(results):
    out = np.empty((TOK, OUT_TOT), dtype=np.float32)
    for d in range(8):
        r, c = divmod(d, C)
        out[r * TOK_SH:(r + 1) * TOK_SH, c * OUT_SH:(c + 1) * OUT_SH] = \
            np.asarray(results[d]["out"]).astype(np.float32)
    return out.reshape(4, 2048, OUT_TOT)


def kernel(x, weight, weight_scale):
    nc = _full_nc()
    in_maps = make_in_maps(x, weight, weight_scale)
    res = run_bass_kernel_spmd(nc, in_maps, core_ids=list(range(8)))
    return assemble(res.results)
